# revision 73
# baseline (speedup 1.0000x reference)
"""IterSpatialCorrelationSampler (P=9, DP=1) Trainium2 Bass kernel.

out[b,i,j,y,x] = sum_c in1[b,c,y,x] * pad(in2)[b,c,y+i,x+j]   (pad=4 each side)

Strategy:
  - 8 cores, each handles (b, yhalf): b = core//2, 48 rows of y.
  - TensorE Gram-band formulation: m-tile = 8y x 16x = 128 output positions
    (PSUM partitions), n = 16x24 = 384 window of padded in2 (free dim),
    contraction over c (256 = 2 accumulating matmuls of k=128).
    psum[(yt,xt), (dy,dx)] = sum_c in1[c, y0+yt, x0+xt] * in2pad[c, y0+dy, x0+dx]
    The 81 useful values per position are psum[(yt,xt), (yt+di, xt+dj)].
  - The matmul moving operand is a strided 2D window AP directly into the
    compact padded in2 SBUF tile (no window materialization copies).
  - Two tx tiles share a 2-bank PSUM tile; one ACT/DVE copy (alternating)
    evacuates both (FD=768 amortizes the fixed per-op cost), keeping the
    evac cadence under the PE's 325 ns/tile so the PE paces the stream.
  - Band staged in SBUF as [p, ty, wy, tx, wx]: the row-extracted slice
    for partition group g (window rows g..g+8 over all tx) is 1728
    contiguous elems, so stores are 3-dim DMA APs with 3456B descriptors.
    Stores go out in waves (after ty1/ty3/ty4/ty5), early waves on the
    otherwise-idle GPSIMD; dma_start costs a flat ~600ns on the issuing
    engine, and a waiting store in a HWDGE FIFO blocks everything behind
    it, so sync's queue carries loads only (plus the final wave).
  - Output wire is 2.65 MB (row-extracted) instead of 4.72 MB (full band);
    host extracts the 81 (di,dj) diagonals from the row-extracted band.
  - PE warm-up dummy matmuls run while the first DMAs land (the HAM clock
    gate reaches 2.4 GHz only after ~3.4us of sustained PE activity);
    tiny "gate" matmuls read SBUF regions that later loads write, so the
    WAR deps hold the bulk loads back and the first tile's inputs get the
    HBM wire to themselves. Filler matmuls at late ty rows bridge input
    stalls so the clock gate stays warm.
  - Inputs cast to fp16 on host. PSUM accumulation is fp32.
"""

import numpy as np

import concourse.bass as bass
import concourse.bacc as bacc
import concourse.tile as tile
import concourse.mybir as mybir
from concourse.bass_utils import run_bass_kernel_spmd

# problem constants (hardcoded per contract)
B, C, H, W = 4, 256, 96, 128
P = 9
OFF = 4
NCORES = 8
YH = H // 2          # 48 rows per core
WP = W + 2 * OFF     # 136
ROWS = YH + 2 * OFF  # 56 rows of padded in2 per core
MT_Y, MT_X = 8, 16   # m-tile shape (8y x 16x = 128 partitions)
NW_Y, NW_X = MT_Y + P - 1, MT_X + P - 1   # 16 x 24 window
NTY, NTX = YH // MT_Y, W // MT_X          # 6 x 8 = 48 tiles
NFREE = NW_Y * NW_X                       # 384
RE = P * NW_X                             # 216 row-extracted elems/partition
NWARM = 6                                 # big PE warm-up dummy matmuls
NFILL = {3: 2, 4: 2, 5: 2}                # PE filler matmuls at late-ty stalls
WAVES = [(0, 2), (2, 4), (4, 5), (5, 6)]  # band store waves (ty ranges)

_cached = {}


def _build():
    nc = bacc.Bacc(
        "TRN2",
        target_bir_lowering=False,
        debug=False,
        enable_asserts=False,
        num_devices=NCORES,
    )
    f16 = mybir.dt.float16
    f32 = mybir.dt.float32

    in1_d = nc.dram_tensor(
        "in1t", [128, NTY, NTX, 2, MT_Y * MT_X], f16, kind="ExternalInput"
    ).ap()
    in2_d = nc.dram_tensor("in2c", [128, 2, ROWS, WP], f16, kind="ExternalInput").ap()
    # [g, lp, ty, di(9 rows), tx, wx] — dim order matches the SBUF source
    # [lp-partitions, ty, rows...] so wave stores are plain 3-dim APs
    band_d = nc.dram_tensor(
        "rband", [MT_Y, NW_Y, NTY, P, NTX, NW_X], f16, kind="ExternalOutput"
    ).ap()

    with tile.TileContext(nc) as tc:
        with (
            tc.tile_pool(name="sb2", bufs=1) as sb2,
            tc.tile_pool(name="ld", bufs=6) as ld,
            tc.tile_pool(name="stage", bufs=3) as stage,
            tc.tile_pool(name="warm", bufs=1) as warm,
            tc.tile_pool(name="ps", bufs=3, space="PSUM") as ps,
            tc.tile_pool(name="psw", bufs=1, space="PSUM") as psw,
        ):
            in2_sb = sb2.tile([128, 2, ROWS, WP], f16)
            # whole-run band staging buffer: [p, ty, wy, tx, wx]; the
            # row-extracted slice (rows g..g+8, all tx, a ty-range) is a
            # 3-dim DMA AP with 1728-elem contiguous runs
            bs = sb2.tile([128, NTY, NW_Y, NTX, NW_X], f16)
            in1_cs = [
                ld.tile([128, NTX, 2, MT_Y * MT_X], f16, tag="in1c", name=f"in1c{i}")
                for i in range(NTY)
            ]

            # PE warm-up: dummy matmuls keep the PE active while input DMAs
            # land (HAM flips to 2.4 GHz after ~3.4us of sustained activity).
            # After the big warmups, tiny (n=128) gate matmuls each READ an
            # SBUF region that a later load writes — the WAR dep holds those
            # loads back so the first compute tile's inputs (in2 rows 0:16 +
            # in1 ty0) get the HBM wire to themselves; the released bulk then
            # round-robins and all chunks land around the same time (~wire
            # end), which is exactly when the last ty rows need them.
            ws = warm.tile([128, 512], f16)
            nc.vector.memset(ws[:, :], 0.0)
            wp = psw.tile([128, 512], f32)
            for _ in range(NWARM):
                nc.tensor.matmul(wp[:, :], ws[:, 0:128], ws[:, :], start=True, stop=True)

            # Chained load release on GPSIMD: each tiny op reads DATA from an
            # already-loaded region (RAW — cannot be hoisted by the
            # scheduler) and also reads the region a later load will write
            # (WAR — holds that load back). Release points: {c1,ty2} after
            # ty0 lands, {c2,ty3} after c1, {ty4,ty5} after ty3 — so at most
            # ~4 DMA queues share the HBM wire and every chunk arrives just
            # before its compute slot.
            gs = warm.tile([128, 24], f16)
            chain = [
                (in2_sb[:, 0, 16, 0:24], in1_cs[0][:, 0, 0, 0:24]),   # c1 <- ty0
                (in1_cs[2][:, 0, 0, 0:24], in1_cs[0][:, 0, 1, 0:24]), # ty2 <- ty0
                (in2_sb[:, 0, 32, 0:24], in2_sb[:, 0, 17, 0:24]),     # c2 <- c1
                (in1_cs[3][:, 0, 0, 0:24], in2_sb[:, 0, 17, 24:48]),  # ty3 <- c1
                (in1_cs[4][:, 0, 0, 0:24], in1_cs[3][:, 0, 1, 0:24]), # ty4 <- ty3
                (in1_cs[5][:, 0, 0, 0:24], in1_cs[3][:, 0, 1, 24:48]),# ty5 <- ty3
            ]
            for region, dep in chain:
                nc.gpsimd.tensor_add(gs[:, :], region, dep)

            # load priority: first compute tile's deps first (ty0+chunk0);
            # the rest are WAR-gated behind the tiny gate matmuls above
            nc.sync.dma_start(out=in2_sb[:, :, 0:16, :], in_=in2_d[:, :, 0:16, :])
            nc.sync.dma_start(out=in1_cs[0][:, :, :, :], in_=in1_d[:, 0, :, :, :])
            nc.sync.dma_start(out=in1_cs[1][:, :, :, :], in_=in1_d[:, 1, :, :, :])
            nc.sync.dma_start(out=in2_sb[:, :, 16:32, :], in_=in2_d[:, :, 16:32, :])
            nc.sync.dma_start(out=in1_cs[2][:, :, :, :], in_=in1_d[:, 2, :, :, :])
            nc.sync.dma_start(out=in2_sb[:, :, 32:ROWS, :], in_=in2_d[:, :, 32:ROWS, :])
            nc.sync.dma_start(out=in1_cs[3][:, :, :, :], in_=in1_d[:, 3, :, :, :])
            nc.sync.dma_start(out=in1_cs[4][:, :, :, :], in_=in1_d[:, 4, :, :, :])
            nc.sync.dma_start(out=in1_cs[5][:, :, :, :], in_=in1_d[:, 5, :, :, :])

            for ty in range(NTY):
                in1_c = in1_cs[ty]
                # filler matmuls: keep PE activity up while waiting for
                # late in1 chunks, so the HAM clock gate stays at 2.4 GHz
                for _ in range(NFILL.get(ty, 0)):
                    nc.tensor.matmul(
                        wp[:, :], ws[:, 0:128], ws[:, :], start=True, stop=True
                    )
                for txp in range(NTX // 2):
                    # two tx tiles share one 2-bank PSUM tile so they can be
                    # evacuated with a single (cheaper per element) copy
                    pt2 = ps.tile([128, 2, 512], f32, tag="pt2")
                    for half in range(2):
                        tx = 2 * txp + half
                        for ch in range(2):
                            nc.tensor.matmul(
                                pt2[:, half, 0:NFREE],
                                in1_c[:, tx, ch, :],
                                in2_sb[
                                    :, ch,
                                    MT_Y * ty : MT_Y * ty + NW_Y,
                                    MT_X * tx : MT_X * tx + NW_X,
                                ],
                                start=(ch == 0),
                                stop=(ch == 1),
                            )
                    if ty == NTY - 1:
                        # last ty: two smaller evacs per pair so the final
                        # evac (which gates the tail stores) lands sooner
                        for half in range(2):
                            tx = 2 * txp + half
                            dst1 = bs[:, ty, :, tx, :]
                            src1 = pt2[:, half, 0:NFREE]
                            if (txp + half) % 2 == 0:
                                nc.scalar.mul(dst1, src1, 1.0)
                            else:
                                nc.vector.tensor_copy(dst1, src1)
                    else:
                        # dst dims [p, wy, tx, wx] -> iterate as [p, tx, wy, wx]
                        dst = bs[:, ty, :, 2 * txp : 2 * txp + 2, :].transpose(
                            [0, 2, 1, 3]
                        )
                        src = pt2[:, :, 0:NFREE]
                        if txp % 2 == 0:
                            nc.scalar.mul(dst, src, 1.0)
                        else:
                            nc.vector.tensor_copy(dst, src)
                # band store waves: one DMA per group g covers the wave's ty
                # range; issue is spread over scalar/sync/gpsimd (flat ~600ns
                # per dma_start on the issuing engine)
                # engine choice: keep compute-dependent stores out of the
                # FIFO of any engine that still has critical work queued
                # (sync is done issuing loads by ty4; ACT must finish ty5
                # evacs before its share of the tail wave).
                for w, (t0, t1) in enumerate(WAVES):
                    if ty != t1 - 1:
                        continue
                    for g in range(MT_Y):
                        # 3-way engine split per wave: per-dma_start issue
                        # costs a flat ~600ns on the issuing engine, so a
                        # single engine serializes a wave to ~5us. ACT takes
                        # the small share early (its queue must get back to
                        # evacs quickly); it leads only in the last wave,
                        # after all evacs are done.
                        if w <= 2:
                            eng = (nc.gpsimd, nc.sync, nc.scalar)[g % 3]
                        else:
                            eng = (nc.scalar, nc.sync, nc.gpsimd)[g % 3]
                        eng.dma_start(
                            out=band_d[g, :, t0:t1, :, :, :],
                            in_=bs[g * 16 : (g + 1) * 16, t0:t1, g : g + P, :, :],
                        )

    nc.compile()
    return nc


def _prep_inputs(input1, input2):
    """Build per-core input maps (fp16, padded, tiled, c split on partitions)."""
    in_maps = []
    pad2 = np.pad(
        np.asarray(input2), ((0, 0), (0, 0), (OFF, OFF), (OFF, OFF))
    )  # [B, C, H+8, WP]
    a1 = np.asarray(input1)
    for core in range(NCORES):
        b, yh = core // 2, core % 2
        y0 = yh * YH
        # in1 tiles: [cp, ty, tx, ch, (my, mx)]
        i1 = a1[b, :, y0 : y0 + YH, :].reshape(2, 128, NTY, MT_Y, NTX, MT_X)
        i1 = i1.transpose(1, 2, 4, 0, 3, 5).reshape(128, NTY, NTX, 2, MT_Y * MT_X)
        # compact padded in2: [cp, ch, rows, cols]
        p2 = pad2[b, :, y0 : y0 + ROWS, :].reshape(2, 128, ROWS, WP)
        i2c = p2.transpose(1, 0, 2, 3).astype(np.float16)  # [128, 2, ROWS, WP]
        in_maps.append(
            {
                "in1t": np.ascontiguousarray(i1.astype(np.float16)),
                "in2c": np.ascontiguousarray(i2c),
            }
        )
    return in_maps


def _extract(rb):
    """rband [MT_Y, NW_Y, NTY, P, NTX, NW_X] f16 -> [9, 9, 48, 128].

    rb[g, lp, ty, di, tx, wx] = band value at window row (g+di), col wx
    for position (y = ty*8+g, x = tx*16+lp). Useful wx = lp + dj.
    """
    arr = rb.transpose(2, 0, 1, 3, 4, 5)  # -> [ty, g, lp, di, tx, wx]
    out = np.empty((P, P, YH, W), dtype=np.float32)
    for di in range(P):
        t = arr[:, :, :, di, :, :]  # [ty, g, lp, tx, wx]
        for dj in range(P):
            d = t.diagonal(dj, 2, 4)  # [ty, g, tx, lp(diag)]
            out[di, dj] = d.reshape(YH, W)
    return out


def run(input1, input2, trace=False, **trace_kwargs):
    if "nc" not in _cached:
        _cached["nc"] = _build()
    nc = _cached["nc"]
    in_maps = _prep_inputs(input1, input2)
    res = run_bass_kernel_spmd(
        nc, in_maps, list(range(NCORES)), trace=trace, **trace_kwargs
    )
    out = np.empty((B, P, P, H, W), dtype=np.float32)
    for core in range(NCORES):
        b, yh = core // 2, core % 2
        rb = res.results[core]["rband"]
        out[b, :, :, yh * YH : (yh + 1) * YH, :] = _extract(rb)
    return out, res


def kernel(input1, input2):
    out, _ = run(input1, input2, trace=False)
    return out
